# revision 1
# baseline (speedup 1.0000x reference)
"""AttentiveFP readout kernel for 8 Trainium2 NeuronCores.

Strategy: graph-contiguous sharding of the V=500k nodes across 8 cores
(seg_ids sorted => each graph's nodes contiguous; split at graph
boundaries nearest V/8 multiples). Every graph lives entirely on one
core, so all segment ops are core-local and no collectives are needed.

Per core: graphs are processed in tiles of 128 (partition dim = graph).
Each graph-tile's nodes (<= NSUB*128, host-padded) are streamed through
SBUF once. Segment sum / weighted segment sum are TensorEngine matmuls
against a one-hot node->graph membership matrix built on-device via
iota==segrel compare. The attention-weighted projection uses
  g_repr = (sum_v a_v * x_v) @ Wp.T + bp   (since sum_v a_v = 1)
so the only V-sized matmuls are the K=128 one-hot reductions.
Segment softmax skips the max-subtraction (|z| <~ 12, exp is safe in
fp32). GRU runs per 128-graph tile on-chip.
"""

import numpy as np
from contextlib import ExitStack

import concourse.bass as bass
import concourse.bacc as bacc
import concourse.mybir as mybir
from concourse import tile
from concourse.bass_utils import run_bass_kernel_spmd

F32 = mybir.dt.float32
BF16 = mybir.dt.bfloat16
NP_BF16 = mybir.dt.np(mybir.dt.bfloat16)
AOP = mybir.AluOpType
ACT = mybir.ActivationFunctionType
AX = mybir.AxisListType

NCORES = 8
F = 256
T = 2
GT = 128  # graphs per tile (partition dim)
LAST_RESULT = None


def _build_program(NT_G, NSUB, bl_vals, stage=99):
    """Build the per-core SPMD program. Returns (nc, ctx).
    stage: debug cutoff; 99 = full program."""
    ctx = ExitStack()
    nc = bacc.Bacc("TRN2")
    epsc = nc.alloc_sbuf_tensor("const-f32-eps", [128, 1], F32)
    nc.gpsimd.memset(epsc.ap(), 1e-30)
    blc = []
    for t in range(T):
        bt = nc.alloc_sbuf_tensor(f"const-f32-bl{t}", [128, 1], F32)
        nc.gpsimd.memset(bt.ap(), float(bl_vals[t]))
        blc.append(bt)
    nc.all_engine_barrier()

    nf_d = nc.dram_tensor("nf", [NT_G * NSUB * 128, F], F32, kind="ExternalInput")
    segrel_d = nc.dram_tensor("segrel", [NT_G * 128, NSUB], F32, kind="ExternalInput")
    iota_d = nc.dram_tensor("iota", [128, 128], F32, kind="ExternalInput")
    identb_d = nc.dram_tensor("identb", [128, 128], BF16, kind="ExternalInput")
    identf_d = nc.dram_tensor("identf", [128, 128], F32, kind="ExternalInput")
    ones1_d = nc.dram_tensor("ones1", [1, 128], F32, kind="ExternalInput")
    wlg_d = [nc.dram_tensor(f"wlg{t}", [128, F], BF16, kind="ExternalInput") for t in range(T)]
    wln_d = [nc.dram_tensor(f"wln{t}", [128, F], BF16, kind="ExternalInput") for t in range(T)]
    wpt_d = [nc.dram_tensor(f"wpt{t}", [F, F], BF16, kind="ExternalInput") for t in range(T)]
    wih_d = [nc.dram_tensor(f"wiht{t}", [F, 3 * F], BF16, kind="ExternalInput") for t in range(T)]
    whh_d = [nc.dram_tensor(f"whht{t}", [F, 3 * F], BF16, kind="ExternalInput") for t in range(T)]
    brz_d = [nc.dram_tensor(f"brz{t}", [128, 2 * F], F32, kind="ExternalInput") for t in range(T)]
    bin_d = [nc.dram_tensor(f"bin{t}", [128, F], F32, kind="ExternalInput") for t in range(T)]
    bhn_d = [nc.dram_tensor(f"bhn{t}", [128, F], F32, kind="ExternalInput") for t in range(T)]
    bpb_d = [nc.dram_tensor(f"bpb{t}", [128, F], F32, kind="ExternalInput") for t in range(T)]
    out_d = nc.dram_tensor("out", [NT_G * 128, F], F32, kind="ExternalOutput")

    with tile.TileContext(nc) as tc:
      with tc.sbuf_pool(name="const", bufs=1) as cpool, \
           tc.sbuf_pool(name="work", bufs=2) as wpool, \
           tc.sbuf_pool(name="small", bufs=2) as spool, \
           tc.sbuf_pool(name="scr", bufs=3) as scrpool, \
           tc.sbuf_pool(name="stage", bufs=6) as stpool, \
           tc.psum_pool(name="pacc", bufs=2) as pacc, \
           tc.psum_pool(name="prz", bufs=2) as prz, \
           tc.psum_pool(name="ptiny", bufs=4) as ptiny:

        iota_sb = cpool.tile_from(iota_d[:, :], name="iota_sb")
        identb_sb = cpool.tile_from(identb_d[:, :], name="identb_sb")
        identf_sb = cpool.tile_from(identf_d[:, :], name="identf_sb")
        ones1_sb = cpool.tile_from(ones1_d[:, :], name="ones1_sb")
        wlg_sb = [cpool.tile_from(wlg_d[t][:, :], name=f"wlg_sb{t}") for t in range(T)]
        wln_sb = [cpool.tile_from(wln_d[t][:, :], name=f"wln_sb{t}") for t in range(T)]
        brz_sb = [cpool.tile_from(brz_d[t][:, :], name=f"brz_sb{t}") for t in range(T)]
        bin_sb = [cpool.tile_from(bin_d[t][:, :], name=f"bin_sb{t}") for t in range(T)]
        bhn_sb = [cpool.tile_from(bhn_d[t][:, :], name=f"bhn_sb{t}") for t in range(T)]
        bpb_sb = [cpool.tile_from(bpb_d[t][:, :], name=f"bpb_sb{t}") for t in range(T)]
        # K-chunked weights: [128, 2, N] with chunk k = rows k*128..k*128+128
        wpt_sb, wih_sb, whh_sb = [], [], []
        for t in range(T):
            wp_t = cpool.tile([128, 2, F], BF16, name=f"wp_sb{t}")
            wi_t = cpool.tile([128, 2, 3 * F], BF16, name=f"wi_sb{t}")
            wh_t = cpool.tile([128, 2, 3 * F], BF16, name=f"wh_sb{t}")
            for k in range(2):
                nc.sync.dma_start(wp_t[:, k, :], wpt_d[t][k * 128:(k + 1) * 128, :])
                nc.sync.dma_start(wi_t[:, k, :], wih_d[t][k * 128:(k + 1) * 128, :])
                nc.sync.dma_start(wh_t[:, k, :], whh_d[t][k * 128:(k + 1) * 128, :])
            wpt_sb.append(wp_t)
            wih_sb.append(wi_t)
            whh_sb.append(wh_t)

        for j in range(NT_G):
            segrel_sb = wpool.tile([128, NSUB], F32, name=f"segrel_{j}", tag="segrel")
            nc.sync.dma_start(segrel_sb[:, :], segrel_d[j * 128:(j + 1) * 128, :])
            nf_aug = wpool.tile([128, NSUB, F + 1], BF16, name=f"nfaug_{j}", tag="nfaug")
            Mn = wpool.tile([128, NSUB, 128], BF16, name=f"Mn_{j}", tag="Mn")
            nc.gpsimd.memset(nf_aug[:, :, 0], 1.0)
            ps_g0 = pacc.tile([128, F + 1], F32, name=f"psg0_{j}", tag="acc")
            for s in range(NSUB):
                stg = stpool.tile([128, F], F32, name=f"stg_{j}_{s}", tag="stage")
                r0 = (j * NSUB + s) * 128
                nc.sync.dma_start(stg[:, :], nf_d[r0:r0 + 128, :])
                if s % 2 == 0:
                    nc.vector.tensor_copy(nf_aug[:, s, 1:F + 1], stg[:, :])
                else:
                    nc.scalar.copy(nf_aug[:, s, 1:F + 1], stg[:, :])
                nc.vector.tensor_tensor(
                    Mn[:, s, :], segrel_sb[:, s:s + 1].broadcast_to((128, 128)),
                    iota_sb[:, :], op=AOP.is_equal)
                nc.tensor.matmul(ps_g0[:, 0:F], Mn[:, s, :],
                                 nf_aug[:, s, 1:F + 1],
                                 start=(s == 0), stop=(s == NSUB - 1))
            # per-node logits' node part: w[t][:, s] = nf . wl_n[t]
            w01 = wpool.tile([128, T, NSUB], F32, name=f"w01_{j}", tag="w01")
            for t in range(T):
                scrw = scrpool.tile([128, NSUB, F], BF16, name=f"scrw_{j}_{t}", tag="scr")
                nc.vector.tensor_tensor(
                    scrw[:, :, :], nf_aug[:, :, 1:F + 1],
                    wln_sb[t][:, :].unsqueeze(1).broadcast_to((128, NSUB, F)),
                    op=AOP.mult)
                nc.vector.reduce_sum(w01[:, t, :], scrw[:, :, :], axis=AX.X)
            gf = spool.tile([128, F], F32, name=f"gf0_{j}", tag="gf", bufs=6)
            nc.scalar.copy(gf[:, :], ps_g0[:, 0:F])

            if stage <= 1:
                nc.sync.dma_start(out_d[j * 128:(j + 1) * 128, :], gf[:, :])
                continue
            for t in range(T):
                # u_g = relu(gf) . wl_g   (per graph), broadcast to nodes
                rgf = spool.tile([128, F], BF16, name=f"rgf_{j}_{t}", tag="rgf")
                nc.scalar.activation(rgf[:, :], gf[:, :], ACT.Relu)
                ucol = spool.tile([128, 1], F32, name=f"ucol_{j}_{t}", tag="ucol")
                uscr = scrpool.tile([128, F], BF16, name=f"uscr_{j}_{t}", tag="uscr")
                nc.vector.tensor_tensor(uscr[:, :], rgf[:, :],
                                        wlg_sb[t][:, :], op=AOP.mult)
                nc.vector.reduce_sum(ucol[:, :], uscr[:, :], axis=AX.X)
                if stage <= 11:
                    nc.vector.tensor_copy(gf[:, 0:1], ucol[:, :])
                    continue
                urow_ps = ptiny.tile([1, 128], F32, name=f"urps_{j}_{t}", tag="tiny")
                nc.tensor.transpose(urow_ps[:, :], ucol[:, :], identf_sb[:, :])
                urow = spool.tile([1, 128], F32, name=f"urow_{j}_{t}", tag="urow")
                nc.scalar.copy(urow[:, :], urow_ps[:, :])
                if stage <= 12:
                    nc.vector.tensor_copy(gf[0:1, :], urow[:, :])
                    continue
                ubc_ps = ptiny.tile([128, 128], F32, name=f"ubcps_{j}_{t}", tag="tiny")
                nc.tensor.matmul(ubc_ps[:, :], ones1_sb[:, :], urow[:, :],
                                 start=True, stop=True)
                ubc = spool.tile([128, 128], BF16, name=f"ubc_{j}_{t}", tag="ubc")
                nc.scalar.copy(ubc[:, :], ubc_ps[:, :])
                if stage <= 13:
                    nc.vector.tensor_copy(gf[:, 0:128], ubc[:, :])
                    continue
                scr2 = scrpool.tile([128, NSUB, 128], BF16, name=f"scr2_{j}_{t}", tag="scr")
                nc.vector.tensor_tensor(
                    scr2[:, :, :], Mn[:, :, :],
                    ubc[:, :].unsqueeze(1).broadcast_to((128, NSUB, 128)),
                    op=AOP.mult)
                ubcv = spool.tile([128, NSUB], F32, name=f"ubcv_{j}_{t}", tag="ubcv")
                nc.vector.reduce_sum(ubcv[:, :], scr2[:, :, :], axis=AX.X)
                if stage <= 14:
                    nc.vector.tensor_copy(gf[:, 0:NSUB], ubcv[:, :])
                    continue
                zt0 = spool.tile([128, NSUB], F32, name=f"zt0_{j}_{t}", tag="zt0")
                nc.vector.tensor_tensor(zt0[:, :], ubcv[:, :], w01[:, t, :],
                                        op=AOP.add)
                zt = spool.tile([128, NSUB], F32, name=f"zt_{j}_{t}", tag="zt")
                nc.vector.tensor_tensor(zt[:, :], zt0[:, :],
                                        blc[t].ap().broadcast_to((128, NSUB)),
                                        op=AOP.add)
                zs = spool.tile([128, NSUB], F32, name=f"zs_{j}_{t}", tag="zs")
                nc.scalar.mul(zs[:, :], zt[:, :], 0.01)
                zl = spool.tile([128, NSUB], F32, name=f"zl_{j}_{t}", tag="zl")
                nc.vector.tensor_tensor(zl[:, :], zt[:, :], zs[:, :], op=AOP.max)
                ebf = spool.tile([128, NSUB], BF16, name=f"ebf_{j}_{t}", tag="ebf")
                nc.scalar.activation(ebf[:, :], zl[:, :], ACT.Exp)
                if stage <= 2:
                    nc.vector.tensor_copy(gf[:, 0:NSUB], ebf[:, :])
                    continue
                # weighted per-node features [e | e*x] and segment-reduce
                scr3 = scrpool.tile([128, NSUB, F + 1], BF16, name=f"scr3_{j}_{t}", tag="scr")
                nc.vector.tensor_tensor(
                    scr3[:, :, :], nf_aug[:, :, :],
                    ebf[:, :].unsqueeze(2).broadcast_to((128, NSUB, F + 1)),
                    op=AOP.mult)
                ps_ds = pacc.tile([128, F + 1], F32, name=f"psds_{j}_{t}", tag="acc")
                for s in range(NSUB):
                    nc.tensor.matmul(ps_ds[:, :], Mn[:, s, :], scr3[:, s, :],
                                     start=(s == 0), stop=(s == NSUB - 1))
                dplus = spool.tile([128, 1], F32, name=f"dplus_{j}_{t}", tag="dplus")
                nc.vector.tensor_tensor(dplus[:, :], ps_ds[:, 0:1], epsc.ap(),
                                        op=AOP.max)
                recd = spool.tile([128, 1], F32, name=f"recd_{j}_{t}", tag="recd")
                nc.vector.reciprocal(recd[:, :], dplus[:, :])
                stl = spool.tile([128, F], BF16, name=f"stl_{j}_{t}", tag="stl")
                nc.vector.tensor_tensor(stl[:, :], ps_ds[:, 1:F + 1],
                                        recd[:, :].broadcast_to((128, F)),
                                        op=AOP.mult)
                if stage <= 3:
                    nc.vector.tensor_copy(gf[:, :], stl[:, :])
                    continue
                # g_repr = stl @ Wp.T  (via transposed stl chunks)
                stT = spool.tile([128, 2, 128], BF16, name=f"stT_{j}_{t}", tag="stT")
                for k in range(2):
                    pst = ptiny.tile([128, 128], BF16, name=f"pst_{j}_{t}_{k}", tag="tiny")
                    nc.tensor.transpose(pst[:, :], stl[:, k * 128:(k + 1) * 128],
                                        identb_sb[:, :])
                    nc.scalar.copy(stT[:, k, :], pst[:, :])
                ps_wp = ptiny.tile([128, F], F32, name=f"pswp_{j}_{t}", tag="tiny")
                for k in range(2):
                    nc.tensor.matmul(ps_wp[:, :], stT[:, k, :], wpt_sb[t][:, k, :],
                                     start=(k == 0), stop=(k == 1))
                # context = elu(g_repr + bp) = relu(x) + exp(min(x,0)) - 1
                xg = spool.tile([128, F], F32, name=f"xg_{j}_{t}", tag="xg")
                nc.vector.tensor_tensor(xg[:, :], ps_wp[:, :], bpb_sb[t][:, :], op=AOP.add)
                xn = spool.tile([128, F], F32, name=f"xn_{j}_{t}", tag="xn")
                nc.vector.tensor_tensor(xn[:, :], xg[:, :],
                                        nc.const_aps.tensor(0.0, (128, F)),
                                        op=AOP.min)
                en = spool.tile([128, F], F32, name=f"en_{j}_{t}", tag="en")
                nc.scalar.activation(en[:, :], xn[:, :], ACT.Exp)
                xp = spool.tile([128, F], F32, name=f"xp_{j}_{t}", tag="xp")
                nc.scalar.activation(xp[:, :], xg[:, :], ACT.Relu)
                s1 = spool.tile([128, F], F32, name=f"s1_{j}_{t}", tag="s1")
                nc.vector.tensor_tensor(s1[:, :], en[:, :], xp[:, :], op=AOP.add)
                ctxb = spool.tile([128, F], BF16, name=f"ctxb_{j}_{t}", tag="ctxb")
                nc.vector.tensor_tensor(ctxb[:, :], s1[:, :],
                                        nc.const_aps.tensor(1.0, (128, F)),
                                        op=AOP.subtract)
                if stage <= 4:
                    nc.vector.tensor_copy(gf[:, :], ctxb[:, :])
                    continue
                # GRU(x=ctxb, h=gf)
                gfb = spool.tile([128, F], BF16, name=f"gfb_{j}_{t}", tag="gfb")
                nc.scalar.copy(gfb[:, :], gf[:, :])
                xT = spool.tile([128, 2, 128], BF16, name=f"xT_{j}_{t}", tag="xT")
                hT = spool.tile([128, 2, 128], BF16, name=f"hT_{j}_{t}", tag="hT")
                for k in range(2):
                    p1 = ptiny.tile([128, 128], BF16, name=f"p1_{j}_{t}_{k}", tag="tiny")
                    nc.tensor.transpose(p1[:, :], ctxb[:, k * 128:(k + 1) * 128],
                                        identb_sb[:, :])
                    nc.scalar.copy(xT[:, k, :], p1[:, :])
                    p2 = ptiny.tile([128, 128], BF16, name=f"p2_{j}_{t}_{k}", tag="tiny")
                    nc.tensor.transpose(p2[:, :], gfb[:, k * 128:(k + 1) * 128],
                                        identb_sb[:, :])
                    nc.scalar.copy(hT[:, k, :], p2[:, :])
                ps_rz = prz.tile([128, 2 * F], F32, name=f"psrz_{j}_{t}", tag="rz")
                mm = 0
                for lhsT, wt in ((xT, wih_sb[t]), (hT, whh_sb[t])):
                    for k in range(2):
                        nc.tensor.matmul(ps_rz[:, :], lhsT[:, k, :],
                                         wt[:, k, 0:2 * F],
                                         start=(mm == 0), stop=(mm == 3))
                        mm += 1
                ps_in = ptiny.tile([128, F], F32, name=f"psin_{j}_{t}", tag="tiny")
                for k in range(2):
                    nc.tensor.matmul(ps_in[:, :], xT[:, k, :],
                                     wih_sb[t][:, k, 2 * F:3 * F],
                                     start=(k == 0), stop=(k == 1))
                ps_hn = ptiny.tile([128, F], F32, name=f"pshn_{j}_{t}", tag="tiny")
                for k in range(2):
                    nc.tensor.matmul(ps_hn[:, :], hT[:, k, :],
                                     whh_sb[t][:, k, 2 * F:3 * F],
                                     start=(k == 0), stop=(k == 1))
                rzs = spool.tile([128, 2 * F], F32, name=f"rzs_{j}_{t}", tag="rzs")
                nc.vector.tensor_tensor(rzs[:, :], ps_rz[:, :], brz_sb[t][:, :], op=AOP.add)
                rza = spool.tile([128, 2 * F], F32, name=f"rza_{j}_{t}", tag="rza")
                nc.scalar.activation(rza[:, :], rzs[:, :], ACT.Sigmoid)
                hns = spool.tile([128, F], F32, name=f"hns_{j}_{t}", tag="hns")
                nc.vector.tensor_tensor(hns[:, :], ps_hn[:, :], bhn_sb[t][:, :], op=AOP.add)
                tmp = spool.tile([128, F], F32, name=f"tmp_{j}_{t}", tag="tmp")
                nc.vector.tensor_tensor(tmp[:, :], rza[:, 0:F], hns[:, :], op=AOP.mult)
                t2 = spool.tile([128, F], F32, name=f"t2_{j}_{t}", tag="t2")
                nc.vector.tensor_tensor(t2[:, :], tmp[:, :], ps_in[:, :], op=AOP.add)
                t3 = spool.tile([128, F], F32, name=f"t3_{j}_{t}", tag="t3")
                nc.vector.tensor_tensor(t3[:, :], t2[:, :], bin_sb[t][:, :], op=AOP.add)
                nn = spool.tile([128, F], F32, name=f"nn_{j}_{t}", tag="nn")
                nc.scalar.activation(nn[:, :], t3[:, :], ACT.Tanh)
                hm = spool.tile([128, F], F32, name=f"hm_{j}_{t}", tag="hm")
                nc.vector.tensor_tensor(hm[:, :], gf[:, :], nn[:, :], op=AOP.subtract)
                hz = spool.tile([128, F], F32, name=f"hz_{j}_{t}", tag="hz")
                nc.vector.tensor_tensor(hz[:, :], hm[:, :], rza[:, F:2 * F], op=AOP.mult)
                gf_new = spool.tile([128, F], F32, name=f"gfn_{j}_{t}", tag="gf", bufs=6)
                nc.vector.tensor_tensor(gf_new[:, :], hz[:, :], nn[:, :], op=AOP.add)
                gf = gf_new
            nc.sync.dma_start(out_d[j * 128:(j + 1) * 128, :], gf[:, :])
    nc.finalize()
    return nc, ctx


def _prep_core(node_feats, seg, g_lo, g_hi, n_lo, n_hi, NT_G, NSUB):
    """Build padded nf / segrel arrays for one core."""
    nf_pad = np.zeros((NT_G * NSUB * 128, F), np.float32)
    segrel = np.full((NT_G * 128, NSUB), -1.0, np.float32)
    for j in range(NT_G):
        gt = g_lo + j * 128
        if gt >= g_hi:
            continue
        ge = min(gt + 128, g_hi)
        a = int(np.searchsorted(seg, gt, 'left'))
        b = int(np.searchsorted(seg, ge, 'left'))
        cnt = b - a
        assert cnt <= NSUB * 128
        nf_pad[j * NSUB * 128: j * NSUB * 128 + cnt] = node_feats[a:b]
        rel = np.full(NSUB * 128, -1.0, np.float32)
        rel[:cnt] = (seg[a:b] - gt).astype(np.float32)
        # segrel[j*128 + p, s] = rel of node s*128+p
        segrel[j * 128:(j + 1) * 128, :] = rel.reshape(NSUB, 128).T
    return nf_pad, segrel


def kernel(node_feats, seg_ids, Wl, bl, Wp, bp, Wih, Whh, bih, bhh):
    node_feats = np.asarray(node_feats, np.float32)
    seg = np.asarray(seg_ids).astype(np.int64)
    Wl = np.asarray(Wl, np.float32)
    bl = np.asarray(bl, np.float32)
    Wp = np.asarray(Wp, np.float32)
    bp = np.asarray(bp, np.float32)
    Wih = np.asarray(Wih, np.float32)
    Whh = np.asarray(Whh, np.float32)
    bih = np.asarray(bih, np.float32)
    bhh = np.asarray(bhh, np.float32)
    V = node_feats.shape[0]
    G = 25000

    # graph-contiguous shard boundaries
    bounds_g = [0]
    for c in range(1, NCORES):
        bounds_g.append(int(seg[c * V // NCORES]))
    bounds_g.append(G)
    bounds_n = [int(np.searchsorted(seg, g, 'left')) for g in bounds_g]

    NT_G = max((bounds_g[c + 1] - bounds_g[c] + 127) // 128 for c in range(NCORES))
    maxnodes = 1
    for c in range(NCORES):
        for gt in range(bounds_g[c], bounds_g[c + 1], 128):
            ge = min(gt + 128, bounds_g[c + 1])
            a = np.searchsorted(seg, gt, 'left')
            b = np.searchsorted(seg, ge, 'left')
            maxnodes = max(maxnodes, int(b - a))
    NSUB = (maxnodes + 127) // 128

    nc, ctx = _build_program(NT_G, NSUB, [float(bl[t, 0]) for t in range(T)])

    # shared (replicated) weight arrays
    shared = {
        "iota": np.broadcast_to(np.arange(128, dtype=np.float32), (128, 128)).copy(),
        "identb": np.eye(128, dtype=np.float32).astype(NP_BF16),
        "identf": np.eye(128, dtype=np.float32),
        "ones1": np.ones((1, 128), np.float32),
    }
    for t in range(T):
        shared[f"wlg{t}"] = np.broadcast_to(Wl[t, 0, :F], (128, F)).astype(NP_BF16)
        shared[f"wln{t}"] = np.broadcast_to(Wl[t, 0, F:], (128, F)).astype(NP_BF16)
        shared[f"wpt{t}"] = Wp[t].T.copy().astype(NP_BF16)
        shared[f"wiht{t}"] = Wih[t].T.copy().astype(NP_BF16)
        shared[f"whht{t}"] = Whh[t].T.copy().astype(NP_BF16)
        shared[f"brz{t}"] = np.broadcast_to(bih[t, :2 * F] + bhh[t, :2 * F], (128, 2 * F)).astype(np.float32).copy()
        shared[f"bin{t}"] = np.broadcast_to(bih[t, 2 * F:], (128, F)).astype(np.float32).copy()
        shared[f"bhn{t}"] = np.broadcast_to(bhh[t, 2 * F:], (128, F)).astype(np.float32).copy()
        shared[f"bpb{t}"] = np.broadcast_to(bp[t], (128, F)).astype(np.float32).copy()

    in_maps = []
    for c in range(NCORES):
        nf_pad, segrel = _prep_core(
            node_feats, seg, bounds_g[c], bounds_g[c + 1],
            bounds_n[c], bounds_n[c + 1], NT_G, NSUB)
        m = dict(shared)
        m["nf"] = nf_pad
        m["segrel"] = segrel
        in_maps.append(m)

    res = run_bass_kernel_spmd(nc, in_maps, core_ids=list(range(NCORES)))
    ctx.close()
    global LAST_RESULT
    LAST_RESULT = res

    out = np.zeros((G, F), np.float32)
    for c in range(NCORES):
        gc = bounds_g[c + 1] - bounds_g[c]
        out[bounds_g[c]:bounds_g[c + 1]] = res.results[c]["out"][:gc]
    return out

